# revision 7
# baseline (speedup 1.0000x reference)
"""AdaptiveRoutingLayer kernel for 8 TRN2 NeuronCores.

Math: out = sum_e softmax(routing_weights[task_id])[e] * (x @ W[e].T + b[e])
The weighted sum over experts is linear, so it collapses to a single matmul:
    out = x @ Wmix.T + bmix,  Wmix = sum_e w[e] * W[e],  bmix = sum_e w[e] * b[e]
Host mixes the weights (cheap: E*D*D MACs), the device does the B x D x D
matmul, data-parallel over the 8 cores (1024 tokens each). No collectives.
"""

import numpy as np
import ml_dtypes

# Problem shapes (hardcoded; kernel.py must be self-contained).
E, T, D, B = 8, 4, 2048, 8192
N_CORES = 8
B_SH = B // N_CORES          # 1024 tokens per core
P = 128                      # SBUF partitions
KT = D // P                  # 16 k-tiles of 128
MT = B_SH // P               # 8 m-tiles of 128 tokens
NTILE = 512                  # matmul free dim (one PSUM bank of fp32)
NT = D // NTILE              # 4 n-tiles

_CACHE = {}


def _build():
    """Build + compile the per-core Bass/Tile graph (same program on all 8 cores)."""
    import concourse.bacc as bacc
    import concourse.mybir as mybir
    import concourse.tile as tile

    nc = bacc.Bacc("TRN2", target_bir_lowering=False, debug=False,
                   num_devices=N_CORES)

    bf16 = mybir.dt.bfloat16
    f32 = mybir.dt.float32

    xT = nc.dram_tensor("xT", [D, B_SH], bf16, kind="ExternalInput").ap()
    wT = nc.dram_tensor("wT", [D, D], bf16, kind="ExternalInput").ap()
    bias = nc.dram_tensor("bias", [P, D], f32, kind="ExternalInput").ap()
    out = nc.dram_tensor("out", [B_SH, D], f32, kind="ExternalOutput").ap()

    HD = D // 2  # 1024: column half of the output / W

    with tile.TileContext(nc) as tc:
        with (
            tc.tile_pool(name="wpool", bufs=1) as wpool,
            tc.tile_pool(name="xpool", bufs=1) as xpool,
            tc.tile_pool(name="bpool", bufs=1) as bpool,
            tc.tile_pool(name="opool", bufs=4) as opool,
            tc.tile_pool(name="pspool", bufs=1, space="PSUM") as pspool,
        ):
            # Whole working set is SBUF-resident: wT 8 MiB + xT 4 MiB + bias 1 MiB.
            # Separate tiles per k-tile so the PE can start as each DMA lands.
            x_tiles = [xpool.tile([P, B_SH], bf16, name=f"x{kt}", tag=f"x{kt}")
                       for kt in range(KT)]
            w_tiles = {}
            for h in range(2):
                for kt in range(KT):
                    w_tiles[(kt, h)] = wpool.tile(
                        [P, HD], bf16, name=f"w{kt}_{h}", tag=f"w{kt}_{h}")
            b_s = bpool.tile([P, D], f32)

            # DMA order = consumption order: (x, w-half-0) per k-tile first
            # (bias early: pass-1 evictions need it), then w-half-1.
            nc.gpsimd.dma_start(b_s[:], bias[:])  # off the sync queue
            for kt in range(KT):
                nc.sync.dma_start(x_tiles[kt][:], xT[kt * P:(kt + 1) * P, :])
                nc.sync.dma_start(w_tiles[(kt, 0)][:], wT[kt * P:(kt + 1) * P, 0:HD])
            for kt in range(KT):
                nc.sync.dma_start(w_tiles[(kt, 1)][:], wT[kt * P:(kt + 1) * P, HD:D])

            # 4 passes x (4 m-tiles x 1024 cols); all 8 PSUM banks live per pass.
            # k-tiles innermost-but-one so pass 1 tracks DMA arrival order.
            for mg, h in ((0, 0), (1, 0), (0, 1), (1, 1)):
                ps = [pspool.tile([P, HD], f32, name=f"ps{mg}{h}{i}", tag=f"ps{i}")
                      for i in range(4)]
                for kt in range(KT):
                    for i in range(4):
                        m = mg * 4 + i
                        lhsT = x_tiles[kt][:, m * P:(m + 1) * P]  # [K=128, M=128]
                        for n2 in range(2):
                            nc.tensor.matmul(
                                ps[i][:, n2 * NTILE:(n2 + 1) * NTILE],
                                lhsT,
                                w_tiles[(kt, h)][:, n2 * NTILE:(n2 + 1) * NTILE],
                                start=(kt == 0),
                                stop=(kt == KT - 1),
                            )
                for i in range(4):
                    m = mg * 4 + i
                    o_t = opool.tile([P, HD], f32, name=f"o{mg}{h}{i}", tag="o")
                    # two 512-col chunks so the out-DMA can start on the first
                    # half while the DVE finishes the second (shorter tail)
                    for n2 in range(2):
                        sl = slice(n2 * NTILE, (n2 + 1) * NTILE)
                        gl = slice(h * HD + n2 * NTILE, h * HD + (n2 + 1) * NTILE)
                        nc.vector.tensor_add(o_t[:, sl], ps[i][:, sl], b_s[:, gl])
                        nc.sync.dma_start(out[m * P:(m + 1) * P, gl], o_t[:, sl])

    nc.compile()
    return nc


def kernel(x, W, b, routing_weights, task_id):
    from concourse.bass_utils import run_bass_kernel_spmd

    tid = int(np.asarray(task_id))
    r = np.asarray(routing_weights, np.float64)[tid]
    w = np.exp(r - r.max())
    w = (w / w.sum()).astype(np.float32)                 # [E]

    Wmix = np.tensordot(w, np.asarray(W, np.float32), axes=([0], [0]))  # [D_out, D_in]
    WmixT = np.ascontiguousarray(Wmix.T).astype(ml_dtypes.bfloat16)     # [D_in, D_out]
    bmix = (w[:, None] * np.asarray(b, np.float32)).sum(0)              # [D]
    bias = np.ascontiguousarray(np.broadcast_to(bmix, (P, D))).astype(np.float32)

    xT = np.asarray(x, np.float32).T.astype(ml_dtypes.bfloat16)         # [D, B]

    if "nc" not in _CACHE:
        _CACHE["nc"] = _build()
    nc = _CACHE["nc"]

    in_maps = [
        {
            "xT": np.ascontiguousarray(xT[:, c * B_SH:(c + 1) * B_SH]),
            "wT": WmixT,
            "bias": bias,
        }
        for c in range(N_CORES)
    ]
    res = run_bass_kernel_spmd(nc, in_maps, core_ids=list(range(N_CORES)))
    return np.concatenate([res.results[c]["out"] for c in range(N_CORES)], axis=0)
